# revision 1
# baseline (speedup 1.0000x reference)
"""GCN (2-layer + pvt projection) Trainium2 kernel, 8-core SPMD. v2.

Changes vs v1 baseline:
- Packed feature tables: pair-row [51200, 128] bf16 where row (t*64+j) of a
  core's shard holds nodes (t*128+j | t*128+64+j). Halves every AllGather and
  table build (features are 64-wide; gather rows must be 256B).
- Deferred W2: gc2 computes A2 = spmm(adj, h1) in 64-wide h-space;
  out = spmm(pvt, A2) @ W2 + rowsum(pvt)*b2^T, then log_softmax. Removes the
  per-tile H2pre matmul+transpose pass; pvt spmm runs mode-T (no full-span).
- AllGathers split per table half (2 x 6.55MB instead of 1 x 25.7MB at v1's
  256B pitch) and pipelined: each half is gathered as soon as the tiles
  feeding it are done, overlapping the next stage's gathers with the tail of
  the current stage.
- Gathers round-robin over 2 SWDGE queues (desc-gen / drain overlap).
"""

import sys

sys.path.insert(0, "/opt/trn_rl_repo")

import numpy as np
import ml_dtypes

from concourse import bass, bacc, mybir, tile
from concourse import bass_utils
from concourse.bass_utils import run_bass_kernel_spmd

# ---- NTFF profiling hook (normally injected by the launcher) -------------


def _install_ntff_hook():
    import types
    import ctypes
    import contextlib

    if "antenv.axon_hooks" in sys.modules:
        return
    hook = None
    so_path = "/opt/axon/libaxon_pjrt.so"
    try:
        lib = ctypes.CDLL(so_path)
        if hasattr(lib, "axon_start_nrt_profile"):
            lib.axon_start_nrt_profile.argtypes = [
                ctypes.POINTER(ctypes.c_int64), ctypes.c_size_t]
            lib.axon_start_nrt_profile.restype = ctypes.c_int64
            lib.axon_stop_nrt_profile.argtypes = [ctypes.c_char_p]
            lib.axon_stop_nrt_profile.restype = ctypes.c_int64

            @contextlib.contextmanager
            def _hook(output_dir, device_ids):
                import jax
                jax.devices()
                if device_ids:
                    ids = (ctypes.c_int64 * len(device_ids))(*device_ids)
                    rc = lib.axon_start_nrt_profile(ids, len(device_ids))
                else:
                    rc = lib.axon_start_nrt_profile(None, 0)
                if rc != 0:
                    raise RuntimeError(f"axon_start_nrt_profile rc={rc}")
                try:
                    yield
                finally:
                    n = lib.axon_stop_nrt_profile(str(output_dir).encode())
                    print(f"ntff profile: {n} file(s) -> {output_dir}")

            hook = _hook
    except OSError:
        pass
    mod = types.ModuleType("antenv.axon_hooks")
    mod.get_axon_ntff_profile_hook = lambda: hook
    mod.set_axon_ntff_profile_hook = lambda h: None
    sys.modules["antenv.axon_hooks"] = mod


_install_ntff_hook()
bass_utils.upload_artifacts = lambda tmpdir: f"local://{tmpdir}"

BF16 = ml_dtypes.bfloat16
NCORES = 8
P = 128
N_LOC = 12800          # local nodes per core (100 tiles)
N_TILES = N_LOC // P   # 100
N_PAD = N_LOC * NCORES  # 102400
LP_LOC = N_LOC // 2    # 6400 local pair rows
LP_HALF = LP_LOC // 2  # 3200 pair rows per half
HROWS = LP_HALF * NCORES  # 25600 table rows per half

FULL = dict(N=100_000, NFEAT=512, NHID=64, NCLASS=40)


# --------------------------------------------------------------------------
# host-side planning
# --------------------------------------------------------------------------

def _src_map(c):
    """Global src node -> (half, table_row, parity)."""
    core_c = c // N_LOC
    lc = c % N_LOC
    t_src = lc >> 7
    j = lc & 127
    p = j >> 6
    lp = t_src * 64 + (j & 63)
    h = (lp >= LP_HALF).astype(np.int64)
    tabrow = core_c * LP_HALF + lp - h * LP_HALF
    return h, tabrow, p


class Plan:
    """(tile, half, parity)-bucketed slot/window layout, core-uniform.

    Windows per bucket: floor(max_e/128) full 128-slot aligned windows plus
    one packed tail window (K = max_e % 128 slots at a partition offset
    inside a shared tail column). Pads only cover cross-core count variance.
    """

    def __init__(self, rows, cols, vals, G_T, bucket_order=None,
                 use_tails=True):
        self.G_T = G_T
        n_groups = N_TILES // G_T
        assert N_TILES % G_T == 0
        self.n_groups = n_groups
        self.groups = [list(range(g * G_T, (g + 1) * G_T))
                       for g in range(n_groups)]
        if bucket_order is None:
            bucket_order = [(0, 0), (0, 1), (1, 0), (1, 1)]
        self.bucket_order = bucket_order

        core = rows // N_LOC
        per = []
        counts = np.zeros((NCORES, N_TILES, 2, 2), np.int64)
        border = np.zeros((2, 2), np.int64)
        for i, (h, p) in enumerate(bucket_order):
            border[h, p] = i
        for k in range(NCORES):
            m = core == k
            d = (rows[m] - k * N_LOC).astype(np.int64)
            c = cols[m].astype(np.int64)
            v = vals[m].astype(np.float32)
            t = d >> 7
            h, tr, p = _src_map(c)
            o = np.lexsort((d, border[h, p], t))
            t, h, p, tr, d, v = t[o], h[o], p[o], tr[o], d[o], v[o]
            np.add.at(counts[k], (t, h, p), 1)
            per.append((t, h, p, tr, d, v))

        maxe = counts.max(axis=0)  # [t, h, p]
        empty = maxe.reshape(N_TILES, 4).sum(axis=1) == 0
        maxe[empty, 0, 0] = 1
        if use_tails:
            nfull = maxe // P
            nrem = maxe % P
        else:
            nfull = -(-maxe // P)
            nrem = np.zeros_like(maxe)

        # ---- region layout: bulk windows then packed tail columns ----
        # tails straddle-split across shared 128-slot columns (1-2 windows)
        slot_base = np.zeros((N_TILES, 2, 2), np.int64)   # bulk start
        tb1 = np.zeros((N_TILES, 2, 2), np.int64)   # tail seg 1 slot start
        tk1 = np.zeros((N_TILES, 2, 2), np.int64)   # tail seg 1 count
        tb2 = np.zeros((N_TILES, 2, 2), np.int64)   # tail seg 2 slot start
        win_base = np.zeros((N_TILES, 2, 2), np.int64)    # global window id
        ntail = np.zeros((N_TILES, 2, 2), np.int64)  # tail windows (0/1/2)
        self.region_off = {}   # (g,h,p) -> (slot_off, n_slots)
        off = 0
        nwin_total = 0
        for g in range(n_groups):
            for h, p in bucket_order:
                b = off
                for t in self.groups[g]:
                    slot_base[t, h, p] = off
                    off += nfull[t, h, p] * P
                # tails from multiple buckets share packed 128-slot columns
                # (straddle-split into <=2 pieces). Their matmuls use the
                # full column (pbase 0, K=128): other buckets' rows are zero
                # in this window's band block, so they contribute nothing.
                fill = P
                for t in self.groups[g]:
                    r = int(nrem[t, h, p])
                    if r == 0:
                        continue
                    if fill == P:
                        col_start = off
                        off += P
                        fill = 0
                    k1 = min(r, P - fill)
                    tb1[t, h, p] = col_start + fill
                    tk1[t, h, p] = k1
                    ntail[t, h, p] = 1
                    fill += k1
                    if fill == P and k1 < r:
                        col_start = off
                        off += P
                        fill = r - k1
                        tb2[t, h, p] = col_start
                        ntail[t, h, p] = 2
                self.region_off[(g, h, p)] = (b, off - b)
                for t in self.groups[g]:
                    win_base[t, h, p] = nwin_total
                    nwin_total += int(nfull[t, h, p] + ntail[t, h, p])
        S = off
        self.S = S

        # ---- per-core slot/window assignment ----
        idx_s = np.zeros((NCORES, S), np.int16)
        val_s = np.zeros((NCORES, S), np.float32)
        pos_s = np.zeros((NCORES, S), np.int64)
        valid = np.zeros((NCORES, S), bool)
        win_of = np.zeros((NCORES, S), np.int64)
        for k in range(NCORES):
            t, h, p, tr, d, v = per[k]
            key = ((t * 2 + h) * 2 + p)
            ne = len(key)
            if ne:
                starts = np.r_[0, np.nonzero(np.diff(key))[0] + 1]
                run_id = np.zeros(ne, np.int64)
                run_id[starts[1:]] = 1
                run_id = np.cumsum(run_id)
                rank = np.arange(ne) - starts[run_id]
                nb = nfull[t, h, p] * P
                rt = rank - nb
                s = np.where(
                    rank < nb, slot_base[t, h, p] + rank,
                    np.where(rt < tk1[t, h, p], tb1[t, h, p] + rt,
                             tb2[t, h, p] + rt - tk1[t, h, p]))
                w = win_base[t, h, p] + np.where(
                    rank < nb, rank // P,
                    np.where(rt < tk1[t, h, p], nfull[t, h, p],
                             nfull[t, h, p] + 1))
                idx_s[k, s] = tr.astype(np.int16)
                val_s[k, s] = v
                pos_s[k, s] = d & 127
                valid[k, s] = True
                win_of[k, s] = w

        # ---- per-window spans (union over cores) ----
        lo = np.full(nwin_total, P, np.int64)
        hi = np.zeros(nwin_total, np.int64)
        for k in range(NCORES):
            m = valid[k]
            np.minimum.at(lo, win_of[k, m], pos_s[k, m])
            np.maximum.at(hi, win_of[k, m], pos_s[k, m] + 1)
        none = hi == 0
        lo[none] = 0
        hi[none] = 1

        # ---- per-tile window lists ----
        # window tuple: (h, p, colF, lhs_off, M, lo, pbase, K)
        self.tile_windows = {}
        lhs_off_w = np.zeros(nwin_total, np.int64)
        lhs_off = 0
        for g in range(n_groups):
            for t in self.groups[g]:
                cis = []
                for h, p in bucket_order:
                    ro = self.region_off[(g, h, p)][0]
                    for c in range(nfull[t, h, p]):
                        cis.append((h, p,
                                    (slot_base[t, h, p] + c * P - ro) // P,
                                    win_base[t, h, p] + c))
                    # tail pieces: full-column matmul (pbase 0, K=128);
                    # band zeros mask other buckets' rows in the column
                    if ntail[t, h, p] >= 1:
                        cis.append((h, p, (tb1[t, h, p] - ro) // P,
                                    win_base[t, h, p] + nfull[t, h, p]))
                    if ntail[t, h, p] == 2:
                        cis.append((h, p, (tb2[t, h, p] - ro) // P,
                                    win_base[t, h, p] + nfull[t, h, p] + 1))
                assert cis, f"tile {t} has no windows"
                # first window full-span: start=True must zero the whole
                # 128-dest psum row for coverage (later windows accumulate)
                lo[cis[0][3]], hi[cis[0][3]] = 0, P
                lst = []
                for h, p, colF, wid in cis:
                    M = int(hi[wid] - lo[wid])
                    lst.append((h, p, int(colF), lhs_off, M,
                                int(lo[wid]), 0, P))
                    lhs_off_w[wid] = lhs_off
                    lhs_off += M
                self.tile_windows[t] = lst
        self.L = lhs_off

        # ---- per-core band + idx tensors ----
        self.lhs_np = []
        self.idx_np = []
        for k in range(NCORES):
            m = valid[k]
            sl = np.nonzero(m)[0]
            lhs = np.zeros((P, self.L), np.float32)
            rowi = sl % P
            wids = win_of[k, sl]
            coli = lhs_off_w[wids] + pos_s[k, sl] - lo[wids]
            assert (coli >= 0).all() and (coli < self.L).all()
            lhs[rowi, coli] = val_s[k, sl]
            self.lhs_np.append(lhs.astype(BF16))
            idx16 = np.zeros((16, S // 16), np.int16)
            ss = np.arange(S)
            idx16[ss % 16, ss // 16] = idx_s[k]
            self.idx_np.append(np.tile(idx16, (NCORES, 1)))  # [128, S//16]

        self.cmax = int(max(n // P for (_, n) in self.region_off.values()))
        self.imax = int(max(n // 16 for (_, n) in self.region_off.values()))
        self.lgmax = int(max(
            sum(w[4] for t in tl for w in self.tile_windows[t])
            for tl in self.groups))

    def group_lhs_span(self, g):
        tl = self.groups[g]
        o0 = self.tile_windows[tl[0]][0][3]
        last = self.tile_windows[tl[-1]][-1]
        return o0, last[3] + last[4] - o0


# --------------------------------------------------------------------------
# numpy emulation of the device dataflow (fast host-side correctness check)
# --------------------------------------------------------------------------

def emulate(plan, tab_full, k):
    """tab_full: [2*HROWS, 128] float32 packed table (both halves).
    Returns [N_LOC, 64] result of spmm for core k per the window schedule."""
    out = np.zeros((N_TILES, 64, P), np.float32)
    idx16 = plan.idx_np[k][:16]
    ss = np.arange(plan.S)
    idx_flat = idx16[ss % 16, ss // 16].astype(np.int64)
    lhs = plan.lhs_np[k].astype(np.float32)
    for g in range(plan.n_groups):
        fbs = {}
        for h in range(2):
            for p in range(2):
                soff, n = plan.region_off[(g, h, p)]
                rows = idx_flat[soff:soff + n] + h * HROWS
                gathered = tab_full[rows]  # [n, 128]
                fbs[(h, p)] = gathered.reshape(n // P, P, P)  # [col][slot][el]
        for t in plan.groups[g]:
            acc = np.zeros((64, P), np.float32)
            for (h, p, colF, loff, M, lo, pb, K) in plan.tile_windows[t]:
                fb = fbs[(h, p)][colF][pb:pb + K, p * 64:p * 64 + 64]
                band = lhs[pb:pb + K, loff:loff + M]  # [K, M]
                acc[:, lo:lo + M] += fb.T @ band
            out[t] = acc
    return out  # [tiles, 64 feats, 128 nodes]


def pack_table(feat):
    """feat: [N_PAD, 64] -> packed [2*HROWS, 128] in table row order."""
    f = feat.reshape(NCORES, N_TILES, 2, 64, 64)  # [core, t, p, j, feat]
    # pair row (core, lp=t*64+j): [feat(p=0) | feat(p=1)]
    pr = np.concatenate((f[:, :, 0], f[:, :, 1]), axis=3)  # [core,t,j,128]
    # [core, h, LP_HALF, 128] -> [h, core, LP_HALF, 128]
    pr = pr.reshape(NCORES, 2, LP_HALF, P).transpose(1, 0, 2, 3)
    return pr.reshape(-1, P)  # [2*HROWS, 128]


# --------------------------------------------------------------------------
# device kernel builder
# --------------------------------------------------------------------------

def build_kernel(ep, pp):
    NFEAT, NHID, NCLASS = FULL["NFEAT"], FULL["NHID"], FULL["NCLASS"]
    ncc = NFEAT // P
    f32 = mybir.dt.float32
    bf16 = mybir.dt.bfloat16
    i16 = mybir.dt.int16

    nc = bacc.Bacc("TRN2", target_bir_lowering=False, debug=False,
                   enable_asserts=False, num_devices=NCORES,
                   num_swdge_queues=2)

    x_d = nc.dram_tensor("x", [N_LOC, NFEAT], bf16, kind="ExternalInput")
    w1_d = nc.dram_tensor("w1", [NFEAT, NHID], bf16, kind="ExternalInput")
    w2_d = nc.dram_tensor("w2", [NHID, NCLASS], bf16, kind="ExternalInput")
    b1_d = nc.dram_tensor("b1", [NHID, 1], f32, kind="ExternalInput")
    b2_d = nc.dram_tensor("b2r", [1, NCLASS], bf16, kind="ExternalInput")
    s_d = nc.dram_tensor("srow", [1, N_LOC], bf16, kind="ExternalInput")
    eidx_d = nc.dram_tensor("eidx", [P, ep.S // 16], i16, kind="ExternalInput")
    elhs_d = nc.dram_tensor("elhs", [P, ep.L], bf16, kind="ExternalInput")
    pidx_d = nc.dram_tensor("pidx", [P, pp.S // 16], i16, kind="ExternalInput")
    plhs_d = nc.dram_tensor("plhs", [P, pp.L], bf16, kind="ExternalInput")
    out_d = nc.dram_tensor("out", [N_LOC, NCLASS], f32, kind="ExternalOutput")

    rg = [list(range(NCORES))]
    qrr = [0]

    def qnext():
        q = qrr[0]
        qrr[0] = (q + 1) % 2
        return q

    with tile.TileContext(nc) as tc:
        with (
            tc.tile_pool(name="dram", bufs=1, space="DRAM") as dram,
            tc.tile_pool(name="const", bufs=1) as cpool,
            tc.tile_pool(name="xt", bufs=8) as xtp,
            tc.tile_pool(name="fbuf", bufs=6) as fpool,
            tc.tile_pool(name="lhsb", bufs=2) as lpool,
            tc.tile_pool(name="stg", bufs=4) as spool,
            tc.tile_pool(name="sfx", bufs=4) as fxpool,
            tc.tile_pool(name="psS", bufs=4, space="PSUM") as pspool,
            tc.tile_pool(name="psP", bufs=2, space="PSUM") as pkpool,
            tc.tile_pool(name="psO", bufs=2, space="PSUM") as ps2pool,
        ):
            ag1_in = dram.tile([LP_LOC, P], bf16, tag="ag1_in")
            ag2_in = dram.tile([LP_LOC, P], bf16, tag="ag2_in")
            ag3_in = dram.tile([LP_LOC, P], bf16, tag="ag3_in")
            tab1 = nc.dram_tensor("tab1", [2 * HROWS, P], bf16,
                                  kind="Internal", addr_space="Shared")
            tab2 = nc.dram_tensor("tab2", [2 * HROWS, P], bf16,
                                  kind="Internal", addr_space="Shared")
            tab3 = nc.dram_tensor("tab3", [2 * HROWS, P], bf16,
                                  kind="Internal", addr_space="Shared")

            # ---- constants ----
            w1_sb = cpool.tile([P, ncc, NHID], bf16, tag="w1")
            nc.sync.dma_start(
                out=w1_sb[:],
                in_=w1_d.ap().rearrange("(c p) f -> p c f", p=P))
            w2_sb = cpool.tile([NHID, NCLASS], bf16, tag="w2")
            nc.sync.dma_start(out=w2_sb[:], in_=w2_d.ap())
            b1_sb = cpool.tile([NHID, 1], f32, tag="b1")
            nc.sync.dma_start(out=b1_sb[:], in_=b1_d.ap())
            b2_sb = cpool.tile([1, NCLASS], bf16, tag="b2")
            nc.sync.dma_start(out=b2_sb[:], in_=b2_d.ap())
            s_sb = cpool.tile([1, N_LOC], bf16, tag="srow")
            nc.sync.dma_start(out=s_sb[:], in_=s_d.ap())
            ident = cpool.tile([64, 64], bf16, tag="ident")
            from concourse.masks import make_identity
            make_identity(nc, ident[:])
            eidx_sb = cpool.tile([P, ep.S // 16], i16, tag="eidx")
            nc.sync.dma_start(out=eidx_sb[:], in_=eidx_d.ap())
            pidx_sb = cpool.tile([P, pp.S // 16], i16, tag="pidx")
            nc.sync.dma_start(out=pidx_sb[:], in_=pidx_d.ap())

            def ag_half(ag_in, tab, h):
                # collectives are Pool-engine-only (BIR checkValidEngines
                # rejects SP/ACT), so the completion-wait unavoidably blocks
                # the in-order gather stream for the AllGather's duration
                nc.gpsimd.collective_compute(
                    "AllGather", mybir.AluOpType.bypass, replica_groups=rg,
                    ins=[ag_in[h * LP_HALF:(h + 1) * LP_HALF, :].opt()],
                    outs=[tab.ap()[h * HROWS:(h + 1) * HROWS, :].opt()])

            def stage_packed(ag_in, t, pk):
                """pk: [64, 128] psum (packed pair rows) -> ag_in rows."""
                stg = spool.tile([64, P], bf16, tag="stg")
                nc.vector.tensor_copy(out=stg[:], in_=pk[:])
                nc.sync.dma_start(
                    out=ag_in[t * 64:(t + 1) * 64, :], in_=stg[:])

            # ---- stage A: XW1 packed table (4 quarter-waves) ----
            for w in range(4):
                trange = range(w * 25, (w + 1) * 25)
                r0 = w * 25 * P
                xts = []
                for cc in range(ncc):
                    xt = xtp.tile([P, 25 * P], bf16, tag="xt")
                    nc.sync.dma_start_transpose(
                        out=xt[:],
                        in_=x_d.ap()[r0:r0 + 25 * P, cc * P:(cc + 1) * P])
                    xts.append(xt)
                for t in trange:
                    pk = pspool.tile([64, P], f32, tag="ps")
                    for half in range(2):
                        c0 = t * P + half * 64 - r0
                        for cc in range(ncc):
                            nc.tensor.matmul(
                                out=pk[:, half * 64:half * 64 + 64],
                                lhsT=xts[cc][:, c0:c0 + 64],
                                rhs=w1_sb[:, cc, :],
                                start=(cc == 0), stop=(cc == ncc - 1))
                    stage_packed(ag1_in, t, pk)
                if w == 1:
                    ag_half(ag1_in, tab1, 0)
                if w == 3:
                    ag_half(ag1_in, tab1, 1)

            # ---- generic SpMM pass (mode T, nf=64) ----
            # Emission is software-pipelined: the first two buckets'
            # (phase-A) gathers run LAG groups ahead of each group's
            # completion (phase-B gathers + matmuls), so the in-order gpsimd
            # queue has runnable gathers while a phase-B AllGather lands.
            LAG = 0

            def spmm(plan, tab, idx_sb, lhs_d, consume, ag_after=None,
                     group_order=None):
                if group_order is None:
                    group_order = list(range(plan.n_groups))

                def gather(g, h, p):
                    soff, n_idx = plan.region_off[(g, h, p)]
                    fb = fpool.tile([P, plan.cmax, P], bf16, tag="F")
                    nc.gpsimd.dma_gather(
                        fb[:, 0:n_idx // P, :],
                        tab.ap()[h * HROWS:(h + 1) * HROWS, :],
                        idx_sb[:, soff // 16:(soff + n_idx) // 16],
                        n_idx, n_idx, P, elem_step=P,
                        single_packet=False, queue_num=qnext())
                    return fb

                fbs_a = {}

                def complete(gi, g):
                    fbs = fbs_a.pop(g)
                    for h, p in plan.bucket_order[2:]:
                        fbs[(h, p)] = gather(g, h, p)
                    o0, lg = plan.group_lhs_span(g)
                    lsb = lpool.tile([P, plan.lgmax], bf16, tag="lhs")
                    nc.sync.dma_start(out=lsb[:, 0:lg],
                                      in_=lhs_d.ap()[:, o0:o0 + lg])
                    for t in plan.groups[g]:
                        wins = plan.tile_windows[t]
                        ps = pspool.tile([64, P], f32, tag="ps")
                        prev = None
                        for i, (h, p, colF, loff, M, lo, pb, K) \
                                in enumerate(wins):
                            mm = nc.tensor.matmul(
                                out=ps[:, lo:lo + M],
                                lhsT=fbs[(h, p)][pb:pb + K, colF,
                                                 p * 64:p * 64 + 64],
                                rhs=lsb[pb:pb + K, loff - o0:loff - o0 + M],
                                start=(i == 0), stop=(i == len(wins) - 1))
                            if prev is not None:
                                tile.add_dep_helper(mm.ins, prev.ins,
                                                    sync=False,
                                                    reason="acc order")
                            prev = mm
                        consume(t, ps)
                    if ag_after and gi in ag_after:
                        ag_after[gi]()

                for gi, g in enumerate(group_order):
                    fbs_a[g] = {(h, p): gather(g, h, p)
                                for h, p in plan.bucket_order[:2]}
                    if gi >= LAG:
                        complete(gi - LAG, group_order[gi - LAG])
                for gi in range(max(0, len(group_order) - LAG),
                                len(group_order)):
                    complete(gi, group_order[gi])

            def pack_consume(ag_in, act):
                """act(t, ps) -> hT sbuf [64,128]; pack + stage."""
                def f(t, ps):
                    hT = act(t, ps)
                    pk = pkpool.tile([64, P], bf16, tag="pk")
                    nc.tensor.transpose(out=pk[:, 0:64], in_=hT[:, 0:64],
                                        identity=ident[:])
                    nc.tensor.transpose(out=pk[:, 64:128], in_=hT[:, 64:128],
                                        identity=ident[:])
                    stage_packed(ag_in, t, pk)
                return f

            # ---- gc1: h1 = relu(spmm(adj, XW1) + b1) ----
            def gc1_act(t, ps):
                hT = spool.tile([NHID, P], bf16, tag="hT")
                nc.scalar.activation(
                    out=hT[:], in_=ps[:],
                    func=mybir.ActivationFunctionType.Relu,
                    bias=b1_sb[:, 0:1], scale=1.0)
                return hT

            ag2s = {ep.n_groups // 2 - 1: lambda: ag_half(ag2_in, tab2, 0),
                    ep.n_groups - 1: lambda: ag_half(ag2_in, tab2, 1)}
            spmm(ep, tab1, eidx_sb, elhs_d, pack_consume(ag2_in, gc1_act),
                 ag_after=ag2s)

            # ---- gc2: A2 = spmm(adj, h1) (no bias; W2 deferred) ----
            def gc2_act(t, ps):
                hT = spool.tile([NHID, P], bf16, tag="hT")
                nc.vector.tensor_copy(out=hT[:], in_=ps[:])
                return hT

            # gc2 runs dest tiles 50-99 first so tab3's h1 half lands early
            # (pvt consumes h1 first); AG3-h0 is covered by pvt's h1 work.
            ag3s = {ep.n_groups // 2 - 1: lambda: ag_half(ag3_in, tab3, 1),
                    ep.n_groups - 1: lambda: ag_half(ag3_in, tab3, 0)}
            rev_order = (list(range(ep.n_groups // 2, ep.n_groups))
                         + list(range(ep.n_groups // 2)))
            spmm(ep, tab2, eidx_sb, elhs_d, pack_consume(ag3_in, gc2_act),
                 ag_after=ag3s, group_order=rev_order)

            # ---- pvt spmm + W2/b2 + per-tile log_softmax ----
            def pvt_consume(t, ps):
                hb = spool.tile([NHID, P], bf16, tag="hT")
                nc.vector.tensor_copy(out=hb[:], in_=ps[:])
                ps2 = ps2pool.tile([P, NCLASS], f32, tag="ps2")
                mm1 = nc.tensor.matmul(out=ps2[:], lhsT=hb[:], rhs=w2_sb[:],
                                       start=True, stop=False)
                mm2 = nc.tensor.matmul(
                    out=ps2[:], lhsT=s_sb[0:1, t * P:(t + 1) * P],
                    rhs=b2_sb[:], start=False, stop=True)
                tile.add_dep_helper(mm2.ins, mm1.ins, sync=False,
                                    reason="acc order")
                mxt = fxpool.tile([P, 1], f32, tag="mxt")
                nc.vector.tensor_reduce(out=mxt[:], in_=ps2[:],
                                        axis=mybir.AxisListType.X,
                                        op=mybir.AluOpType.max)
                sh = fxpool.tile([P, NCLASS], f32, tag="sh")
                nc.vector.tensor_scalar(
                    out=sh[:], in0=ps2[:], scalar1=mxt[:, 0:1], scalar2=None,
                    op0=mybir.AluOpType.subtract)
                eb = fxpool.tile([P, NCLASS], f32, tag="eb")
                st = fxpool.tile([P, 1], f32, tag="st")
                nc.scalar.activation(out=eb[:], in_=sh[:],
                                     func=mybir.ActivationFunctionType.Exp,
                                     accum_out=st[:, 0:1])
                lst = fxpool.tile([P, 1], f32, tag="lst")
                nc.scalar.activation(out=lst[:], in_=st[:],
                                     func=mybir.ActivationFunctionType.Ln)
                ob = fxpool.tile([P, NCLASS], f32, tag="ob")
                nc.vector.tensor_scalar(
                    out=ob[:], in0=sh[:], scalar1=lst[:, 0:1], scalar2=None,
                    op0=mybir.AluOpType.subtract)
                nc.sync.dma_start(out=out_d.ap()[t * P:(t + 1) * P, :],
                                  in_=ob[:])

            spmm(pp, tab3, pidx_sb, plhs_d, pvt_consume)

    nc.compile()
    return nc


# --------------------------------------------------------------------------
# host driver
# --------------------------------------------------------------------------

USE_TAILS = True


def _plan(inputs, G_T_adj=5, G_T_pvt=10):
    ep = Plan(np.asarray(inputs["adj_row"]).astype(np.int64),
              np.asarray(inputs["adj_col"]).astype(np.int64),
              np.asarray(inputs["adj_val"], np.float32), G_T_adj,
              use_tails=USE_TAILS)
    pp = Plan(np.asarray(inputs["pvt_row"]).astype(np.int64),
              np.asarray(inputs["pvt_col"]).astype(np.int64),
              np.asarray(inputs["pvt_val"], np.float32), G_T_pvt,
              bucket_order=[(1, 0), (1, 1), (0, 0), (0, 1)],
              use_tails=USE_TAILS)
    return ep, pp


def _run(inputs, dims=None, G_T=5, trace=True, plans=None):
    NFEAT, NHID, NCLASS = FULL["NFEAT"], FULL["NHID"], FULL["NCLASS"]
    N = FULL["N"]
    if plans is None:
        plans = _plan(inputs)
    ep, pp = plans

    nc = build_kernel(ep, pp)

    x_pad = np.zeros((N_PAD, NFEAT), BF16)
    x_pad[:N] = np.asarray(inputs["x"], np.float32).astype(BF16)
    w1 = np.asarray(inputs["W1"], np.float32).astype(BF16)
    w2 = np.asarray(inputs["W2"], np.float32).astype(BF16)
    b1 = np.asarray(inputs["b1"], np.float32).reshape(NHID, 1)
    b2r = np.asarray(inputs["b2"], np.float32).reshape(1, NCLASS).astype(BF16)

    s_full = np.zeros(N_PAD, np.float32)
    np.add.at(s_full, np.asarray(inputs["pvt_row"]).astype(np.int64),
              np.asarray(inputs["pvt_val"], np.float32))

    in_maps = []
    for k in range(NCORES):
        in_maps.append({
            "x": x_pad[k * N_LOC:(k + 1) * N_LOC],
            "w1": w1, "w2": w2, "b1": b1, "b2r": b2r,
            "srow": s_full[k * N_LOC:(k + 1) * N_LOC]
                    .reshape(1, N_LOC).astype(BF16),
            "eidx": ep.idx_np[k], "elhs": ep.lhs_np[k],
            "pidx": pp.idx_np[k], "plhs": pp.lhs_np[k],
        })

    res = run_bass_kernel_spmd(nc, in_maps, core_ids=list(range(NCORES)),
                               trace=trace)
    _run.last_exec_time_ns = res.exec_time_ns
    out = np.concatenate([r["out"] for r in res.results], axis=0)[:N]
    return np.ascontiguousarray(out.astype(np.float32))


_run.last_exec_time_ns = None


def kernel(**inputs) -> np.ndarray:
    return _run(inputs)

